# revision 17
# baseline (speedup 1.0000x reference)
"""Trainium2 Bass kernel for the autoregressive LSTM decoder problem.

Full-input contract: kernel(**inputs) takes the unsharded numpy inputs
(B=8192, D=512, K=24) and returns (out1, out2), each [B, K] float32.

Strategy (data-parallel over 8 NeuronCores, B/8 = 1024 batch per core):
  * State kept transposed on-chip: h,c as [D, B_shard]; the per-step gate
    matmul z^T = Wu^T h^T + Wx^T x^T lands in PSUM gate-major.
  * The dominant matmul runs in fp8 e4m3 DoubleRow mode (2 contraction
    rows/cycle): Wu and h are stored pre-scaled (x32 / x8, powers of two)
    as fp8 pairs [128, 2, *]; the combined x256 scale is undone for free
    by the activation's scale field. First-step error compensation: the
    t=0 h comes with an fp8 residual tensor (host-prepped) and steps 0-2
    also accumulate h @ Wu_residual — this kills the early-step error
    spikes that out2's small scale would otherwise amplify.
  * PSUM per wave (k-tile, half-batch): i|f|o in one 3-bank span (single
    spanned Sigmoid evacuation) + g in its own bank (Tanh). A rank-1
    bf16 closer (x@Wx, exact-ish; gate bias is structurally zero) closes
    each accumulation group.
  * All elementwise state math is fp16 on VectorE (2x packed mode); h is
    also down-converted to the fp8 pair layout for the next step.
  * The y heads (dense1/dense2) for step t-1 run at the START of step t
    from the fp16 h — this removes the y-dependency from the step tail;
    y1's sigmoid feeds back as the bf16 closer input x. y2's elu is
    deferred and applied once, batched [K, B_shard], after the loop.
  * ScalarE (activations, 1 elem/cycle/lane) is the structural roofline
    at ~22us/step; PE (~12us) and VectorE (~17us) hide under it.
"""

import contextlib
import sys

import numpy as np

for _p in ("/opt/trn_rl_repo", "/root/.axon_site/_ro/trn_rl_repo"):
    if _p not in sys.path:
        sys.path.append(_p)

import concourse.bass as bass
import concourse.mybir as mybir
from concourse.tile import TileContext
from concourse.vector_clock import ScopedClock

F32 = mybir.dt.float32
F16 = mybir.dt.float16
BF16 = mybir.dt.bfloat16
FP8 = mybir.dt.float8e4
AF = mybir.ActivationFunctionType
DR = mybir.MatmulPerfMode.DoubleRow

D = 512
B = 1024          # batch per core
NCORES = 8
K = 24
G = 4 * D         # 2048 gate rows
N = 512           # psum bank width (fp32)
SH = 8.0          # h fp8 scale
SW = 32.0         # Wu fp8 scale
SC = SH * SW      # psum scale, undone in the activation
N_WLO = 3         # steps accumulating the Wu fp8-residual term

_MAX_WAITS_PER_DRAIN = 1


def _split_waits(nc):
    """The walrus build in this container accepts at most one semaphore wait
    per instruction. Rebuild every basic block, hoisting all-but-one wait of
    any overloaded instruction onto same-engine InstEventSemaphore
    instructions inserted immediately before it — the engine blocks at the
    same program point for the same conditions, so this is
    semantics-preserving."""
    n_new = 0
    for f in nc.m.functions:
        for blk in f.blocks:
            insts = list(blk.instructions)
            out = []
            changed = False
            for inst in insts:
                si = inst.sync_info
                waits = list(si.on_wait) if si is not None else []
                if len(waits) > 1:
                    changed = True
                    excess, keep = waits[:-1], waits[-1:]
                    for w in excess:
                        ev = mybir.InstEventSemaphore(
                            name=f"splitw-{n_new}", ins=[], outs=[],
                            engine=inst.engine,
                        )
                        ev.sync_info = mybir.SyncInfo(on_wait=[w], on_update=[])
                        nc.register_instruction(ev, overwrite=True)
                        out.append(ev)
                        n_new += 1
                    inst.sync_info = mybir.SyncInfo(
                        on_wait=keep, on_update=list(si.on_update)
                    )
                out.append(inst)
            if changed:
                blk.instructions = out
    return n_new


class SplitDrainTileContext(TileContext):
    """The walrus build in this container rejects Drain (CTRL_NO)
    instructions carrying more than ~2 sync waits; split the tail drain's
    waits across a chain of Drain instructions, one wait each."""

    def _drain_and_barrier(self, tick_clock, wait_clock):
        nc = self.nc
        drain_inst = nc.sync.drain()
        wait_clock.add_sem_waits(
            drain_inst.ins, ScopedClock({None: tick_clock.global_clock})
        )
        si = drain_inst.ins.sync_info
        waits = list(si.on_wait) if si is not None else []
        if len(waits) > _MAX_WAITS_PER_DRAIN:
            drain_inst.ins.sync_info = mybir.SyncInfo(
                on_wait=waits[:_MAX_WAITS_PER_DRAIN], on_update=[]
            )
            for i in range(_MAX_WAITS_PER_DRAIN, len(waits), _MAX_WAITS_PER_DRAIN):
                extra = nc.sync.drain()
                extra.ins.sync_info = mybir.SyncInfo(
                    on_wait=waits[i : i + _MAX_WAITS_PER_DRAIN], on_update=[]
                )

        nc.all_engine_barrier()
        assert self.sems is not None
        popped = nc._tile_sem_poison_stack.pop()
        assert popped is self._sem_poison
        nc.clear_and_free_semaphores(list(self.sems.allocated().values()))
        nc.all_engine_barrier()


def build_nc(repeat: int = 0):
    """repeat=0: straight-line kernel. repeat>=1: whole body wrapped in a
    For_i loop run `repeat` times (only used for timing measurements)."""
    nc = bass.Bass()

    wu_hi = nc.dram_tensor("wu_hi", [128, 2, 2, G], FP8, kind="ExternalInput")
    wu_lo = nc.dram_tensor("wu_lo", [128, 2, 2, G], FP8, kind="ExternalInput")
    wx = nc.dram_tensor("wx", [1, G], BF16, kind="ExternalInput")
    w12 = nc.dram_tensor("w12", [128, 4, 2], F16, kind="ExternalInput")
    h0hi = nc.dram_tensor("h0hi", [128, 2, 2, B], FP8, kind="ExternalInput")
    h0lo = nc.dram_tensor("h0lo", [128, 2, 2, B], FP8, kind="ExternalInput")
    c0 = nc.dram_tensor("c0", [128, 4, B], F16, kind="ExternalInput")
    x0 = nc.dram_tensor("x0", [1, B], BF16, kind="ExternalInput")
    b12 = nc.dram_tensor("b12", [2, 1], F32, kind="ExternalInput")
    b2col = nc.dram_tensor("b2col", [K, 1], F32, kind="ExternalInput")
    ys1 = nc.dram_tensor("ys1", [K, B], F32, kind="ExternalOutput")
    ys2 = nc.dram_tensor("ys2", [K, B], F32, kind="ExternalOutput")

    with SplitDrainTileContext(nc) as tc:
        with contextlib.ExitStack() as ctx:
            wpool = ctx.enter_context(tc.tile_pool(name="w", bufs=1))
            hpool = ctx.enter_context(tc.tile_pool(name="h8", bufs=2))
            hlopool = ctx.enter_context(tc.tile_pool(name="h8lo", bufs=1))
            h16pool = ctx.enter_context(tc.tile_pool(name="h16", bufs=2))
            cpool = ctx.enter_context(tc.tile_pool(name="c", bufs=2))
            gpool = ctx.enter_context(tc.tile_pool(name="g", bufs=5))
            tpool = ctx.enter_context(tc.tile_pool(name="t", bufs=3))
            xpool = ctx.enter_context(tc.tile_pool(name="x", bufs=2))
            ypool = ctx.enter_context(tc.tile_pool(name="y", bufs=2))
            opool = ctx.enter_context(tc.tile_pool(name="o", bufs=1))
            zifo = ctx.enter_context(tc.tile_pool(name="zifo", bufs=2, space="PSUM"))
            zgp = ctx.enter_context(tc.tile_pool(name="zg", bufs=1, space="PSUM"))
            yps = ctx.enter_context(tc.tile_pool(name="yp", bufs=1, space="PSUM"))

            loop_cm = tc.For_i(0, repeat) if repeat else contextlib.nullcontext()
            with loop_cm:
                # --- weights + state init -------------------------------
                wu_hi_sb = wpool.tile([128, 2, 2, G], FP8, tag="wuhi")
                nc.sync.dma_start(wu_hi_sb[:, :, :, :], wu_hi[:, :, :, :])
                wu_lo_sb = wpool.tile([128, 2, 2, G], FP8, tag="wulo")
                nc.sync.dma_start(wu_lo_sb[:, :, :, :], wu_lo[:, :, :, :])
                wx_sb = wpool.tile([1, G], BF16, tag="wx")
                nc.sync.dma_start(wx_sb[0:1, :], wx[0:1, :])
                w12_sb = wpool.tile([128, 4, 2], F16, tag="w12")
                nc.sync.dma_start(w12_sb[:, :, :], w12[:, :, :])
                b12_sb = wpool.tile([2, 1], F32, tag="b12")
                nc.sync.dma_start(b12_sb[:, :], b12[:, :])
                b2c_sb = wpool.tile([K, 1], F32, tag="b2col")
                nc.sync.dma_start(b2c_sb[:, :], b2col[:, :])

                hp_prev = {}
                hp0_lo = {}
                for j in range(2):
                    hp = hpool.tile([128, 2, B], FP8, tag=f"h8_{j}")
                    nc.sync.dma_start(hp[:, :, :], h0hi[:, j, :, :])
                    hp_prev[j] = hp
                    hl = hlopool.tile([128, 2, B], FP8, tag=f"h8lo_{j}")
                    nc.sync.dma_start(hl[:, :, :], h0lo[:, j, :, :])
                    hp0_lo[j] = hl
                c_prev = {}
                for k in range(4):
                    ct = cpool.tile([128, B], F16, tag=f"c_{k}")
                    nc.sync.dma_start(ct[:, :], c0[:, k, :])
                    c_prev[k] = ct
                x_t = xpool.tile([1, B], BF16, tag="x")
                nc.sync.dma_start(x_t[0:1, :], x0[0:1, :])

                ys2pre = opool.tile([K, B], F32, tag="ys2pre")
                h16_prev = {}

                def yhead(tprev, h16, x_dst):
                    """dense1/dense2 matmuls + y1 sigmoid for step tprev; y1
                    also feeds back (bf16) into x_dst's row 0 + 3 row copies."""
                    for n in range(2):
                        ns = slice(n * N, (n + 1) * N)
                        yp = yps.tile([2, N], F32, tag="y")
                        for k in range(4):
                            nc.tensor.matmul(
                                yp[:, :], w12_sb[:, k, :], h16[k][:, ns],
                                start=(k == 0), stop=(k == 3),
                            )
                        yr1 = ypool.tile([1, N], F32, tag="yr1")
                        nc.scalar.activation(
                            yr1[:, :], yp[0:1, :], AF.Sigmoid,
                            bias=b12_sb[0:1, 0:1],
                        )
                        nc.sync.dma_start(ys1[tprev:tprev + 1, ns], yr1[:, :])
                        if x_dst is not None:
                            nc.vector.tensor_copy(x_dst[0:1, ns], yr1[:, :])
                        yr2 = ypool.tile([2, N], F32, tag="yr2")
                        nc.vector.tensor_copy(yr2[:, :], yp[0:2, :])
                        nc.sync.dma_start(ys2pre[tprev:tprev + 1, ns], yr2[1:2, :])

                # --- decode steps ---------------------------------------
                for t in range(K):
                    if t > 0:
                        x_t = xpool.tile([1, B], BF16, tag="x")
                        yhead(t - 1, h16_prev, x_t)
                    h16_new, c_new, hp_new = {}, {}, {}
                    for k in range(4):
                        mi, mf, mo, mg = k, 4 + k, 12 + k, 8 + k
                        o_tiles = {}
                        cn = cpool.tile([128, B], F16, tag=f"c_{k}")
                        nsl = [slice(0, N), slice(N, 2 * N)]
                        # both batch halves' PSUM tiles live at once so every
                        # weight tile is consumed by two back-to-back matmuls
                        # (the PE skips the ~180ns stationary reload when
                        # consecutive matmuls share weights)
                        zifo_n = [zifo.tile([128, 3 * N], F32, tag="zifo",
                                            name=f"zifo_{n}") for n in range(2)]
                        ifo_ms = [(mi, 0), (mf, 1), (mo, 2)]
                        for m, q in ifo_ms:
                            ms = slice(m * 128, (m + 1) * 128)
                            cs = slice(q * N, (q + 1) * N)
                            for j in range(2):
                                for n in range(2):
                                    nc.tensor.matmul(
                                        zifo_n[n][:, cs], wu_hi_sb[:, j, :, ms],
                                        hp_prev[j][:, :, nsl[n]],
                                        start=(j == 0), stop=False, perf_mode=DR,
                                    )
                            if t == 0:
                                for j in range(2):
                                    for n in range(2):
                                        nc.tensor.matmul(
                                            zifo_n[n][:, cs], wu_hi_sb[:, j, :, ms],
                                            hp0_lo[j][:, :, nsl[n]],
                                            start=False, stop=False, perf_mode=DR,
                                        )
                            if t < N_WLO:
                                for j in range(2):
                                    for n in range(2):
                                        nc.tensor.matmul(
                                            zifo_n[n][:, cs], wu_lo_sb[:, j, :, ms],
                                            hp_prev[j][:, :, nsl[n]],
                                            start=False, stop=False, perf_mode=DR,
                                        )
                        # rank-1 x closers (bf16), n-paired per weight row.
                        # Gate bias is structurally zero (asserted host-side).
                        for m, q in ifo_ms:
                            ms = slice(m * 128, (m + 1) * 128)
                            cs = slice(q * N, (q + 1) * N)
                            for n in range(2):
                                nc.tensor.matmul(
                                    zifo_n[n][:, cs], wx_sb[0:1, ms],
                                    x_t[0:1, nsl[n]], start=False, stop=True,
                                )
                        # g gate: single zg bank, n-serial (tanh frees it)
                        gt = {}
                        msg = slice(mg * 128, (mg + 1) * 128)
                        for n in range(2):
                            zg_t = zgp.tile([128, N], F32, tag="zg")
                            for j in range(2):
                                nc.tensor.matmul(
                                    zg_t[:, :], wu_hi_sb[:, j, :, msg],
                                    hp_prev[j][:, :, nsl[n]],
                                    start=(j == 0), stop=False, perf_mode=DR,
                                )
                                if t == 0:
                                    nc.tensor.matmul(
                                        zg_t[:, :], wu_hi_sb[:, j, :, msg],
                                        hp0_lo[j][:, :, nsl[n]],
                                        start=False, stop=False, perf_mode=DR,
                                    )
                                if t < N_WLO:
                                    nc.tensor.matmul(
                                        zg_t[:, :], wu_lo_sb[:, j, :, msg],
                                        hp_prev[j][:, :, nsl[n]],
                                        start=False, stop=False, perf_mode=DR,
                                    )
                            nc.tensor.matmul(
                                zg_t[:, :], wx_sb[0:1, msg], x_t[0:1, nsl[n]],
                                start=False, stop=True,
                            )
                            g_t = gpool.tile([128, N], F16, tag="g", name=f"g_{n}")
                            nc.scalar.activation(
                                g_t[:, :], zg_t[:, :], AF.Tanh, scale=1.0 / SC
                            )
                            gt[n] = g_t
                        for n in range(2):
                            ns = nsl[n]
                            ifo = gpool.tile([128, 3 * N], F16, tag="ifo",
                                             name=f"ifo_{n}")
                            nc.scalar.activation(
                                ifo[:, :], zifo_n[n][:, :], AF.Sigmoid,
                                scale=1.0 / SC
                            )
                            t2 = tpool.tile([128, N], F16, tag="t2")
                            nc.vector.tensor_mul(t2[:, :], ifo[:, 0:N], gt[n][:, :])
                            t1 = tpool.tile([128, N], F16, tag="t1")
                            nc.vector.tensor_mul(
                                t1[:, :], ifo[:, N:2 * N], c_prev[k][:, ns]
                            )
                            nc.vector.tensor_add(cn[:, ns], t1[:, :], t2[:, :])
                            o_tiles[n] = ifo
                        c_new[k] = cn
                        tch = tpool.tile([128, B], F16, tag="tch")
                        nc.scalar.activation(tch[:, :], cn[:, :], AF.Tanh)
                        hn = h16pool.tile([128, B], F16, tag=f"h16_{k}")
                        for n in range(2):
                            ns = slice(n * N, (n + 1) * N)
                            nc.vector.tensor_mul(
                                hn[:, ns], o_tiles[n][:, 2 * N:3 * N], tch[:, ns]
                            )
                        h16_new[k] = hn
                        if t < K - 1:
                            j, i = k // 2, k % 2
                            if i == 0:
                                hp_new[j] = hpool.tile(
                                    [128, 2, B], FP8, tag=f"h8_{j}", name=f"h8n_{j}"
                                )
                            nc.vector.tensor_scalar_mul(
                                hp_new[j][:, i, :], hn[:, :], SH
                            )
                    h16_prev, c_prev, hp_prev = h16_new, c_new, hp_new

                yhead(K - 1, h16_prev, None)

                # --- batched elu tail: y2 = relu(p) + exp(min(p,0)) - 1 --
                pb = opool.tile([K, B], F32, tag="elu_p")
                nc.scalar.activation(
                    pb[:, :], ys2pre[:, :], AF.Identity, bias=b2c_sb[:, 0:1]
                )
                r = opool.tile([K, B], F32, tag="elu_r")
                nc.scalar.activation(r[:, :], pb[:, :], AF.Relu)
                neg = opool.tile([K, B], F32, tag="elu_n")
                nc.vector.tensor_sub(neg[:, :], pb[:, :], r[:, :])
                e = opool.tile([K, B], F32, tag="elu_e")
                nc.scalar.activation(e[:, :], neg[:, :], AF.Exp)
                s = opool.tile([K, B], F32, tag="elu_s")
                nc.vector.tensor_add(s[:, :], r[:, :], e[:, :])
                y2f = opool.tile([K, B], F32, tag="elu_y")
                nc.vector.tensor_scalar_add(y2f[:, :], s[:, :], -1.0)
                nc.sync.dma_start(ys2[:, :], y2f[:, :])

    _split_waits(nc)
    return nc


def make_in_map(initial, encoder_hidden, encoder_cell, Wx, Wu, b, w1, b1, w2, b2):
    """Per-core input dict from this core's batch shard (numpy fp32 arrays)."""
    import ml_dtypes
    e4 = ml_dtypes.float8_e4m3
    bf = ml_dtypes.bfloat16

    def pair_layout(w_rows):  # [D, F] -> [128, 2, 2, F]  (row d = (2j+i)*128+p)
        f = w_rows.shape[1]
        return np.ascontiguousarray(
            w_rows.reshape(2, 2, 128, f).transpose(2, 0, 1, 3)
        )

    assert not np.any(b), "gate bias must be zero (no bias closers built)"
    Wus = (Wu * SW).astype(np.float32)
    wu_hi = Wus.astype(e4)
    wu_lo = (Wus - wu_hi.astype(np.float32)).astype(e4)

    hT = np.ascontiguousarray(encoder_hidden.T).astype(np.float32)
    hs = hT * SH
    h_hi = hs.astype(e4)
    h_lo = (hs - h_hi.astype(np.float32)).astype(e4)

    cT = np.ascontiguousarray(encoder_cell.T)
    w12f = np.concatenate([w1, w2], axis=1)  # [D, 2]

    return {
        "wu_hi": pair_layout(wu_hi),
        "wu_lo": pair_layout(wu_lo),
        "wx": np.ascontiguousarray((Wx * SC).astype(bf).reshape(1, G)),
        "w12": np.ascontiguousarray(
            w12f.reshape(4, 128, 2).transpose(1, 0, 2)
        ).astype(np.float16),
        "h0hi": pair_layout(h_hi),
        "h0lo": pair_layout(h_lo),
        "c0": np.ascontiguousarray(
            cT.reshape(4, 128, B).transpose(1, 0, 2)
        ).astype(np.float16),
        "x0": np.ascontiguousarray(initial[:, 0, :].T.astype(bf).reshape(1, B)),
        "b12": np.array([[np.float32(b1[0])], [np.float32(b2[0])]], dtype=np.float32),
        "b2col": np.full((K, 1), np.float32(b2[0]), dtype=np.float32),
    }


_CACHE = {}


def _get_nc():
    if "nc" not in _CACHE:
        _CACHE["nc"] = build_nc(repeat=0)
    return _CACHE["nc"]


def kernel(initial, encoder_hidden, encoder_cell, Wx, Wu, b, w1, b1, w2, b2):
    from concourse import bass_utils

    initial = np.asarray(initial, dtype=np.float32)
    encoder_hidden = np.asarray(encoder_hidden, dtype=np.float32)
    encoder_cell = np.asarray(encoder_cell, dtype=np.float32)
    Wx = np.asarray(Wx, dtype=np.float32)
    Wu = np.asarray(Wu, dtype=np.float32)
    b = np.asarray(b, dtype=np.float32)
    w1 = np.asarray(w1, dtype=np.float32)
    b1 = np.asarray(b1, dtype=np.float32)
    w2 = np.asarray(w2, dtype=np.float32)
    b2 = np.asarray(b2, dtype=np.float32)

    nc = _get_nc()
    in_maps = []
    for c in range(NCORES):
        sl = slice(c * B, (c + 1) * B)
        in_maps.append(
            make_in_map(initial[sl], encoder_hidden[sl], encoder_cell[sl],
                        Wx, Wu, b, w1, b1, w2, b2)
        )
    res = bass_utils.run_bass_kernel_spmd(nc, in_maps, core_ids=list(range(NCORES)))
    out1 = np.concatenate([res.results[c]["ys1"].T for c in range(NCORES)], axis=0)
    out2 = np.concatenate([res.results[c]["ys2"].T for c in range(NCORES)], axis=0)
    return (np.ascontiguousarray(out1, dtype=np.float32),
            np.ascontiguousarray(out2, dtype=np.float32))


# revision 21
# speedup vs baseline: 1.0701x; 1.0701x over previous
"""Trainium2 Bass kernel for the autoregressive LSTM decoder problem.

Full-input contract: kernel(**inputs) takes the unsharded numpy inputs
(B=8192, D=512, K=24) and returns (out1, out2), each [B, K] float32.

Strategy (data-parallel over 8 NeuronCores, B/8 = 1024 batch per core):
  * State kept transposed on-chip: h,c as [D, B_shard]; the per-step gate
    matmul z^T = Wu^T h^T + Wx^T x^T lands in PSUM gate-major.
  * The dominant matmul runs in fp8 e4m3 DoubleRow mode (2 contraction
    rows/cycle): Wu and h are stored pre-scaled (x32 / x8, powers of two)
    as fp8 pairs [128, 2, *]; the combined x256 scale is undone for free
    by the activation's scale field. First-step error compensation: the
    t=0 h comes with an fp8 residual tensor (host-prepped) and steps 0-2
    also accumulate h @ Wu_residual — this kills the early-step error
    spikes that out2's small scale would otherwise amplify.
  * PSUM per wave (k-tile, half-batch): i|f|o in one 3-bank span (single
    spanned Sigmoid evacuation) + g in its own bank (Tanh). A rank-1
    bf16 closer (x@Wx, exact-ish; gate bias is structurally zero) closes
    each accumulation group.
  * All elementwise state math is fp16 on VectorE (2x packed mode); h is
    also down-converted to the fp8 pair layout for the next step.
  * The y heads (dense1/dense2) for step t-1 run at the START of step t
    from the fp16 h — this removes the y-dependency from the step tail;
    y1's sigmoid feeds back as the bf16 closer input x. y2's elu is
    deferred and applied once, batched [K, B_shard], after the loop.
  * Measured on hardware, every PE matmul carries ~180ns of stationary
    weight-load overhead the cost model omits (and K=1 closers ~170ns
    fixed), so the PE is the real per-step bound (~34us busy) over
    ScalarE (~22us) and VectorE (~15us). Matmuls that could pair over
    batch halves are emitted adjacently to exploit the hardware's
    skip-reload of an unchanged stationary, though the tile scheduler
    only preserves some of those pairings. PSUM caps matmul outputs at
    one 512-col fp32 bank (bank-crossing outputs are rejected), which
    rules out full-width 1024-col matmuls as a weight-amortization fix.
"""

import contextlib
import sys

import numpy as np

for _p in ("/opt/trn_rl_repo", "/root/.axon_site/_ro/trn_rl_repo"):
    if _p not in sys.path:
        sys.path.append(_p)

import concourse.bass as bass
import concourse.mybir as mybir
from concourse.tile import TileContext
from concourse.vector_clock import ScopedClock

F32 = mybir.dt.float32
F16 = mybir.dt.float16
BF16 = mybir.dt.bfloat16
FP8 = mybir.dt.float8e4
AF = mybir.ActivationFunctionType
DR = mybir.MatmulPerfMode.DoubleRow

D = 512
B = 1024          # batch per core
NCORES = 8
K = 24
G = 4 * D         # 2048 gate rows
N = 512           # psum bank width (fp32)
SH = 8.0          # h fp8 scale
SW = 32.0         # Wu fp8 scale
SC = SH * SW      # psum scale, undone in the activation
N_WLO = 3         # steps accumulating the Wu fp8-residual term

_MAX_WAITS_PER_DRAIN = 1


def _split_waits(nc):
    """The walrus build in this container accepts at most one semaphore wait
    per instruction. Rebuild every basic block, hoisting all-but-one wait of
    any overloaded instruction onto same-engine InstEventSemaphore
    instructions inserted immediately before it — the engine blocks at the
    same program point for the same conditions, so this is
    semantics-preserving."""
    n_new = 0
    for f in nc.m.functions:
        for blk in f.blocks:
            insts = list(blk.instructions)
            out = []
            changed = False
            for inst in insts:
                si = inst.sync_info
                waits = list(si.on_wait) if si is not None else []
                if len(waits) > 1:
                    changed = True
                    excess, keep = waits[:-1], waits[-1:]
                    for w in excess:
                        ev = mybir.InstEventSemaphore(
                            name=f"splitw-{n_new}", ins=[], outs=[],
                            engine=inst.engine,
                        )
                        ev.sync_info = mybir.SyncInfo(on_wait=[w], on_update=[])
                        nc.register_instruction(ev, overwrite=True)
                        out.append(ev)
                        n_new += 1
                    inst.sync_info = mybir.SyncInfo(
                        on_wait=keep, on_update=list(si.on_update)
                    )
                out.append(inst)
            if changed:
                blk.instructions = out
    return n_new


class SplitDrainTileContext(TileContext):
    """The walrus build in this container rejects Drain (CTRL_NO)
    instructions carrying more than ~2 sync waits; split the tail drain's
    waits across a chain of Drain instructions, one wait each."""

    def _drain_and_barrier(self, tick_clock, wait_clock):
        nc = self.nc
        drain_inst = nc.sync.drain()
        wait_clock.add_sem_waits(
            drain_inst.ins, ScopedClock({None: tick_clock.global_clock})
        )
        si = drain_inst.ins.sync_info
        waits = list(si.on_wait) if si is not None else []
        if len(waits) > _MAX_WAITS_PER_DRAIN:
            drain_inst.ins.sync_info = mybir.SyncInfo(
                on_wait=waits[:_MAX_WAITS_PER_DRAIN], on_update=[]
            )
            for i in range(_MAX_WAITS_PER_DRAIN, len(waits), _MAX_WAITS_PER_DRAIN):
                extra = nc.sync.drain()
                extra.ins.sync_info = mybir.SyncInfo(
                    on_wait=waits[i : i + _MAX_WAITS_PER_DRAIN], on_update=[]
                )

        nc.all_engine_barrier()
        assert self.sems is not None
        popped = nc._tile_sem_poison_stack.pop()
        assert popped is self._sem_poison
        nc.clear_and_free_semaphores(list(self.sems.allocated().values()))
        nc.all_engine_barrier()


def build_nc(repeat: int = 0):
    """repeat=0: straight-line kernel. repeat>=1: whole body wrapped in a
    For_i loop run `repeat` times (only used for timing measurements)."""
    nc = bass.Bass()

    wu_hi = nc.dram_tensor("wu_hi", [128, 2, 2, G], FP8, kind="ExternalInput")
    wu_lo = nc.dram_tensor("wu_lo", [128, 2, 2, G], FP8, kind="ExternalInput")
    wx = nc.dram_tensor("wx", [1, G], BF16, kind="ExternalInput")
    w12 = nc.dram_tensor("w12", [128, 4, 2], F16, kind="ExternalInput")
    h0hi = nc.dram_tensor("h0hi", [128, 2, 2, B], FP8, kind="ExternalInput")
    h0lo = nc.dram_tensor("h0lo", [128, 2, 2, B], FP8, kind="ExternalInput")
    c0 = nc.dram_tensor("c0", [128, 4, B], F16, kind="ExternalInput")
    x0 = nc.dram_tensor("x0", [1, B], BF16, kind="ExternalInput")
    b12 = nc.dram_tensor("b12", [2, 1], F32, kind="ExternalInput")
    b2col = nc.dram_tensor("b2col", [K, 1], F32, kind="ExternalInput")
    ys1 = nc.dram_tensor("ys1", [K, B], F32, kind="ExternalOutput")
    ys2 = nc.dram_tensor("ys2", [K, B], F32, kind="ExternalOutput")

    with SplitDrainTileContext(nc) as tc:
        with contextlib.ExitStack() as ctx:
            wpool = ctx.enter_context(tc.tile_pool(name="w", bufs=1))
            hpool = ctx.enter_context(tc.tile_pool(name="h8", bufs=2))
            hlopool = ctx.enter_context(tc.tile_pool(name="h8lo", bufs=1))
            h16pool = ctx.enter_context(tc.tile_pool(name="h16", bufs=2))
            cpool = ctx.enter_context(tc.tile_pool(name="c", bufs=2))
            gpool = ctx.enter_context(tc.tile_pool(name="g", bufs=5))
            tpool = ctx.enter_context(tc.tile_pool(name="t", bufs=3))
            xpool = ctx.enter_context(tc.tile_pool(name="x", bufs=2))
            ypool = ctx.enter_context(tc.tile_pool(name="y", bufs=2))
            opool = ctx.enter_context(tc.tile_pool(name="o", bufs=1))
            zifo = ctx.enter_context(tc.tile_pool(name="zifo", bufs=2, space="PSUM"))
            zgp = ctx.enter_context(tc.tile_pool(name="zg", bufs=1, space="PSUM"))
            yps = ctx.enter_context(tc.tile_pool(name="yp", bufs=1, space="PSUM"))

            loop_cm = tc.For_i(0, repeat) if repeat else contextlib.nullcontext()
            with loop_cm:
                # --- weights + state init -------------------------------
                wu_hi_sb = wpool.tile([128, 2, 2, G], FP8, tag="wuhi")
                nc.sync.dma_start(wu_hi_sb[:, :, :, :], wu_hi[:, :, :, :])
                wu_lo_sb = wpool.tile([128, 2, 2, G], FP8, tag="wulo")
                nc.sync.dma_start(wu_lo_sb[:, :, :, :], wu_lo[:, :, :, :])
                wx_sb = wpool.tile([1, G], BF16, tag="wx")
                nc.sync.dma_start(wx_sb[0:1, :], wx[0:1, :])
                w12_sb = wpool.tile([128, 4, 2], F16, tag="w12")
                nc.sync.dma_start(w12_sb[:, :, :], w12[:, :, :])
                b12_sb = wpool.tile([2, 1], F32, tag="b12")
                nc.sync.dma_start(b12_sb[:, :], b12[:, :])
                b2c_sb = wpool.tile([K, 1], F32, tag="b2col")
                nc.sync.dma_start(b2c_sb[:, :], b2col[:, :])

                hp_prev = {}
                hp0_lo = {}
                for j in range(2):
                    hp = hpool.tile([128, 2, B], FP8, tag=f"h8_{j}")
                    nc.sync.dma_start(hp[:, :, :], h0hi[:, j, :, :])
                    hp_prev[j] = hp
                    hl = hlopool.tile([128, 2, B], FP8, tag=f"h8lo_{j}")
                    nc.sync.dma_start(hl[:, :, :], h0lo[:, j, :, :])
                    hp0_lo[j] = hl
                c_prev = {}
                for k in range(4):
                    ct = cpool.tile([128, B], F16, tag=f"c_{k}")
                    nc.sync.dma_start(ct[:, :], c0[:, k, :])
                    c_prev[k] = ct
                x_t = xpool.tile([1, B], BF16, tag="x")
                nc.sync.dma_start(x_t[0:1, :], x0[0:1, :])

                ys2pre = opool.tile([K, B], F32, tag="ys2pre")
                h16_prev = {}

                def yhead(tprev, h16, x_dst):
                    """dense1/dense2 matmuls + y1 sigmoid for step tprev; y1
                    also feeds back (bf16) into x_dst's row 0 + 3 row copies."""
                    for n in range(2):
                        ns = slice(n * N, (n + 1) * N)
                        yp = yps.tile([2, N], F32, tag="y")
                        for k in range(4):
                            nc.tensor.matmul(
                                yp[:, :], w12_sb[:, k, :], h16[k][:, ns],
                                start=(k == 0), stop=(k == 3),
                            )
                        yr1 = ypool.tile([1, N], F32, tag="yr1")
                        nc.scalar.activation(
                            yr1[:, :], yp[0:1, :], AF.Sigmoid,
                            bias=b12_sb[0:1, 0:1],
                        )
                        nc.sync.dma_start(ys1[tprev:tprev + 1, ns], yr1[:, :])
                        if x_dst is not None:
                            nc.vector.tensor_copy(x_dst[0:1, ns], yr1[:, :])
                        yr2 = ypool.tile([2, N], F32, tag="yr2")
                        nc.vector.tensor_copy(yr2[:, :], yp[0:2, :])
                        nc.sync.dma_start(ys2pre[tprev:tprev + 1, ns], yr2[1:2, :])

                # --- decode steps ---------------------------------------
                for t in range(K):
                    if t > 0:
                        x_t = xpool.tile([1, B], BF16, tag="x")
                        yhead(t - 1, h16_prev, x_t)
                    h16_new, c_new, hp_new = {}, {}, {}
                    for k in range(4):
                        mi, mf, mo, mg = k, 4 + k, 12 + k, 8 + k
                        o_tiles = {}
                        cn = cpool.tile([128, B], F16, tag=f"c_{k}")
                        last = k == 3
                        if last:
                            # k3's tail runs per batch-half, n1 first, so the
                            # cross-step chain (tch -> h16 -> h8/y -> x ->
                            # closers) rides the short n0 half at step end
                            hn = h16pool.tile([128, B], F16, tag=f"h16_{k}")
                        nsl = [slice(0, N), slice(N, 2 * N)]
                        # both batch halves' PSUM tiles live at once so every
                        # weight tile is consumed by two back-to-back matmuls
                        # (the PE skips the ~180ns stationary reload when
                        # consecutive matmuls share weights)
                        zifo_n = [zifo.tile([128, 3 * N], F32, tag="zifo",
                                            name=f"zifo_{n}") for n in range(2)]
                        ifo_ms = [(mi, 0), (mf, 1), (mo, 2)]
                        for m, q in ifo_ms:
                            ms = slice(m * 128, (m + 1) * 128)
                            cs = slice(q * N, (q + 1) * N)
                            for j in range(2):
                                for n in range(2):
                                    nc.tensor.matmul(
                                        zifo_n[n][:, cs], wu_hi_sb[:, j, :, ms],
                                        hp_prev[j][:, :, nsl[n]],
                                        start=(j == 0), stop=False, perf_mode=DR,
                                    )
                            if t == 0:
                                for j in range(2):
                                    for n in range(2):
                                        nc.tensor.matmul(
                                            zifo_n[n][:, cs], wu_hi_sb[:, j, :, ms],
                                            hp0_lo[j][:, :, nsl[n]],
                                            start=False, stop=False, perf_mode=DR,
                                        )
                            if t < N_WLO:
                                for j in range(2):
                                    for n in range(2):
                                        nc.tensor.matmul(
                                            zifo_n[n][:, cs], wu_lo_sb[:, j, :, ms],
                                            hp_prev[j][:, :, nsl[n]],
                                            start=False, stop=False, perf_mode=DR,
                                        )
                        # rank-1 x closers (bf16), n-paired per weight row.
                        # Gate bias is structurally zero (asserted host-side).
                        for m, q in ifo_ms:
                            ms = slice(m * 128, (m + 1) * 128)
                            cs = slice(q * N, (q + 1) * N)
                            for n in range(2):
                                nc.tensor.matmul(
                                    zifo_n[n][:, cs], wx_sb[0:1, ms],
                                    x_t[0:1, nsl[n]], start=False, stop=True,
                                )
                        # g gate: single zg bank, n-serial (tanh frees it)
                        gt = {}
                        msg = slice(mg * 128, (mg + 1) * 128)
                        for n in range(2):
                            zg_t = zgp.tile([128, N], F32, tag="zg")
                            for j in range(2):
                                nc.tensor.matmul(
                                    zg_t[:, :], wu_hi_sb[:, j, :, msg],
                                    hp_prev[j][:, :, nsl[n]],
                                    start=(j == 0), stop=False, perf_mode=DR,
                                )
                                if t == 0:
                                    nc.tensor.matmul(
                                        zg_t[:, :], wu_hi_sb[:, j, :, msg],
                                        hp0_lo[j][:, :, nsl[n]],
                                        start=False, stop=False, perf_mode=DR,
                                    )
                                if t < N_WLO:
                                    nc.tensor.matmul(
                                        zg_t[:, :], wu_lo_sb[:, j, :, msg],
                                        hp_prev[j][:, :, nsl[n]],
                                        start=False, stop=False, perf_mode=DR,
                                    )
                            nc.tensor.matmul(
                                zg_t[:, :], wx_sb[0:1, msg], x_t[0:1, nsl[n]],
                                start=False, stop=True,
                            )
                            g_t = gpool.tile([128, N], F16, tag="g", name=f"g_{n}")
                            nc.scalar.activation(
                                g_t[:, :], zg_t[:, :], AF.Tanh, scale=1.0 / SC
                            )
                            gt[n] = g_t
                        for n in ((1, 0) if last else (0, 1)):
                            ns = nsl[n]
                            ifo = gpool.tile([128, 3 * N], F16, tag="ifo",
                                             name=f"ifo_{n}")
                            nc.scalar.activation(
                                ifo[:, :], zifo_n[n][:, :], AF.Sigmoid,
                                scale=1.0 / SC
                            )
                            t2 = tpool.tile([128, N], F16, tag="t2")
                            nc.vector.tensor_mul(t2[:, :], ifo[:, 0:N], gt[n][:, :])
                            t1 = tpool.tile([128, N], F16, tag="t1")
                            nc.vector.tensor_mul(
                                t1[:, :], ifo[:, N:2 * N], c_prev[k][:, ns]
                            )
                            nc.vector.tensor_add(cn[:, ns], t1[:, :], t2[:, :])
                            o_tiles[n] = ifo
                            if last:
                                tch3 = tpool.tile([128, N], F16, tag="tch3")
                                nc.scalar.activation(tch3[:, :], cn[:, ns], AF.Tanh)
                                nc.vector.tensor_mul(
                                    hn[:, ns], ifo[:, 2 * N:3 * N], tch3[:, :]
                                )
                                if t < K - 1:
                                    nc.vector.tensor_scalar_mul(
                                        hp_new[1][:, 1, ns], hn[:, ns], SH
                                    )
                        c_new[k] = cn
                        if not last:
                            tch = tpool.tile([128, B], F16, tag="tch")
                            nc.scalar.activation(tch[:, :], cn[:, :], AF.Tanh)
                            hn = h16pool.tile([128, B], F16, tag=f"h16_{k}")
                            for n in range(2):
                                ns = slice(n * N, (n + 1) * N)
                                nc.vector.tensor_mul(
                                    hn[:, ns], o_tiles[n][:, 2 * N:3 * N], tch[:, ns]
                                )
                            if t < K - 1:
                                j, i = k // 2, k % 2
                                if i == 0:
                                    hp_new[j] = hpool.tile(
                                        [128, 2, B], FP8, tag=f"h8_{j}",
                                        name=f"h8n_{j}"
                                    )
                                nc.vector.tensor_scalar_mul(
                                    hp_new[j][:, i, :], hn[:, :], SH
                                )
                        h16_new[k] = hn
                    h16_prev, c_prev, hp_prev = h16_new, c_new, hp_new

                yhead(K - 1, h16_prev, None)

                # --- batched elu tail: y2 = relu(p) + exp(min(p,0)) - 1 --
                pb = opool.tile([K, B], F32, tag="elu_p")
                nc.scalar.activation(
                    pb[:, :], ys2pre[:, :], AF.Identity, bias=b2c_sb[:, 0:1]
                )
                r = opool.tile([K, B], F32, tag="elu_r")
                nc.scalar.activation(r[:, :], pb[:, :], AF.Relu)
                neg = opool.tile([K, B], F32, tag="elu_n")
                nc.vector.tensor_sub(neg[:, :], pb[:, :], r[:, :])
                e = opool.tile([K, B], F32, tag="elu_e")
                nc.scalar.activation(e[:, :], neg[:, :], AF.Exp)
                s = opool.tile([K, B], F32, tag="elu_s")
                nc.vector.tensor_add(s[:, :], r[:, :], e[:, :])
                y2f = opool.tile([K, B], F32, tag="elu_y")
                nc.vector.tensor_scalar_add(y2f[:, :], s[:, :], -1.0)
                nc.sync.dma_start(ys2[:, :], y2f[:, :])

    _split_waits(nc)
    return nc


def make_in_map(initial, encoder_hidden, encoder_cell, Wx, Wu, b, w1, b1, w2, b2):
    """Per-core input dict from this core's batch shard (numpy fp32 arrays)."""
    import ml_dtypes
    e4 = ml_dtypes.float8_e4m3
    bf = ml_dtypes.bfloat16

    def pair_layout(w_rows):  # [D, F] -> [128, 2, 2, F]  (row d = (2j+i)*128+p)
        f = w_rows.shape[1]
        return np.ascontiguousarray(
            w_rows.reshape(2, 2, 128, f).transpose(2, 0, 1, 3)
        )

    assert not np.any(b), "gate bias must be zero (no bias closers built)"
    Wus = (Wu * SW).astype(np.float32)
    wu_hi = Wus.astype(e4)
    wu_lo = (Wus - wu_hi.astype(np.float32)).astype(e4)

    hT = np.ascontiguousarray(encoder_hidden.T).astype(np.float32)
    hs = hT * SH
    h_hi = hs.astype(e4)
    h_lo = (hs - h_hi.astype(np.float32)).astype(e4)

    cT = np.ascontiguousarray(encoder_cell.T)
    w12f = np.concatenate([w1, w2], axis=1)  # [D, 2]

    return {
        "wu_hi": pair_layout(wu_hi),
        "wu_lo": pair_layout(wu_lo),
        "wx": np.ascontiguousarray((Wx * SC).astype(bf).reshape(1, G)),
        "w12": np.ascontiguousarray(
            w12f.reshape(4, 128, 2).transpose(1, 0, 2)
        ).astype(np.float16),
        "h0hi": pair_layout(h_hi),
        "h0lo": pair_layout(h_lo),
        "c0": np.ascontiguousarray(
            cT.reshape(4, 128, B).transpose(1, 0, 2)
        ).astype(np.float16),
        "x0": np.ascontiguousarray(initial[:, 0, :].T.astype(bf).reshape(1, B)),
        "b12": np.array([[np.float32(b1[0])], [np.float32(b2[0])]], dtype=np.float32),
        "b2col": np.full((K, 1), np.float32(b2[0]), dtype=np.float32),
    }


_CACHE = {}


def _get_nc():
    if "nc" not in _CACHE:
        _CACHE["nc"] = build_nc(repeat=0)
    return _CACHE["nc"]


def kernel(initial, encoder_hidden, encoder_cell, Wx, Wu, b, w1, b1, w2, b2):
    from concourse import bass_utils

    initial = np.asarray(initial, dtype=np.float32)
    encoder_hidden = np.asarray(encoder_hidden, dtype=np.float32)
    encoder_cell = np.asarray(encoder_cell, dtype=np.float32)
    Wx = np.asarray(Wx, dtype=np.float32)
    Wu = np.asarray(Wu, dtype=np.float32)
    b = np.asarray(b, dtype=np.float32)
    w1 = np.asarray(w1, dtype=np.float32)
    b1 = np.asarray(b1, dtype=np.float32)
    w2 = np.asarray(w2, dtype=np.float32)
    b2 = np.asarray(b2, dtype=np.float32)

    nc = _get_nc()
    in_maps = []
    for c in range(NCORES):
        sl = slice(c * B, (c + 1) * B)
        in_maps.append(
            make_in_map(initial[sl], encoder_hidden[sl], encoder_cell[sl],
                        Wx, Wu, b, w1, b1, w2, b2)
        )
    res = bass_utils.run_bass_kernel_spmd(nc, in_maps, core_ids=list(range(NCORES)))
    out1 = np.concatenate([res.results[c]["ys1"].T for c in range(NCORES)], axis=0)
    out2 = np.concatenate([res.results[c]["ys2"].T for c in range(NCORES)], axis=0)
    return (np.ascontiguousarray(out1, dtype=np.float32),
            np.ascontiguousarray(out2, dtype=np.float32))


# revision 25
# speedup vs baseline: 1.1246x; 1.0510x over previous
"""Trainium2 Bass kernel for the autoregressive LSTM decoder problem.

Full-input contract: kernel(**inputs) takes the unsharded numpy inputs
(B=8192, D=512, K=24) and returns (out1, out2), each [B, K] float32.

Strategy (data-parallel over 8 NeuronCores, B/8 = 1024 batch per core):
  * State kept transposed on-chip: h,c as [D, B_shard]; the per-step gate
    matmul z^T = Wu^T h^T + Wx^T x^T lands in PSUM gate-major.
  * The dominant matmul runs in fp8 e4m3 DoubleRow mode (2 contraction
    rows/cycle): Wu and h are stored pre-scaled (x32 / x8, powers of two)
    as fp8 pairs [128, 2, *]; the combined x256 scale is undone for free
    by the activation's scale field. First-step error compensation: the
    t=0 h comes with an fp8 residual tensor (host-prepped) and steps 0-2
    also accumulate h @ Wu_residual — this kills the early-step error
    spikes that out2's small scale would otherwise amplify.
  * PSUM per wave (k-tile, half-batch): i|f|o in one 3-bank span (single
    spanned Sigmoid evacuation) + g in its own bank (Tanh). A rank-1
    bf16 closer (x@Wx, exact-ish; gate bias is structurally zero) closes
    each accumulation group.
  * All elementwise state math is fp16 on VectorE (2x packed mode); h is
    also down-converted to the fp8 pair layout for the next step.
  * The y heads (dense1/dense2) for step t-1 run at the START of step t
    from the fp16 h — this removes the y-dependency from the step tail;
    y1's sigmoid feeds back as the bf16 closer input x. y2's elu is
    deferred and applied once, batched [K, B_shard], after the loop.
  * Measured on hardware, every PE matmul carries ~180ns of stationary
    weight-load overhead the cost model omits (and K=1 closers ~170ns
    fixed), so the PE is the real per-step bound (~34us busy) over
    ScalarE (~22us) and VectorE (~15us). Matmuls that could pair over
    batch halves are emitted adjacently to exploit the hardware's
    skip-reload of an unchanged stationary, though the tile scheduler
    only preserves some of those pairings. PSUM caps matmul outputs at
    one 512-col fp32 bank (bank-crossing outputs are rejected), which
    rules out full-width 1024-col matmuls as a weight-amortization fix.
"""

import contextlib
import sys

import numpy as np

for _p in ("/opt/trn_rl_repo", "/root/.axon_site/_ro/trn_rl_repo"):
    if _p not in sys.path:
        sys.path.append(_p)

import concourse.bass as bass
import concourse.mybir as mybir
from concourse.tile import TileContext
from concourse.vector_clock import ScopedClock

F32 = mybir.dt.float32
F16 = mybir.dt.float16
BF16 = mybir.dt.bfloat16
FP8 = mybir.dt.float8e4
AF = mybir.ActivationFunctionType
DR = mybir.MatmulPerfMode.DoubleRow

D = 512
B = 1024          # batch per core
NCORES = 8
K = 24
G = 4 * D         # 2048 gate rows
N = 512           # psum bank width (fp32)
SH = 8.0          # h fp8 scale
SW = 32.0         # Wu fp8 scale
SC = SH * SW      # psum scale, undone in the activation
N_WLO = 3         # steps accumulating the Wu fp8-residual term

_MAX_WAITS_PER_DRAIN = 1


def _split_waits(nc):
    """The walrus build in this container accepts at most one semaphore wait
    per instruction. Rebuild every basic block, hoisting all-but-one wait of
    any overloaded instruction onto same-engine InstEventSemaphore
    instructions inserted immediately before it — the engine blocks at the
    same program point for the same conditions, so this is
    semantics-preserving."""
    n_new = 0
    for f in nc.m.functions:
        for blk in f.blocks:
            insts = list(blk.instructions)
            out = []
            changed = False
            for inst in insts:
                si = inst.sync_info
                waits = list(si.on_wait) if si is not None else []
                if len(waits) > 1:
                    changed = True
                    excess, keep = waits[:-1], waits[-1:]
                    for w in excess:
                        ev = mybir.InstEventSemaphore(
                            name=f"splitw-{n_new}", ins=[], outs=[],
                            engine=inst.engine,
                        )
                        ev.sync_info = mybir.SyncInfo(on_wait=[w], on_update=[])
                        nc.register_instruction(ev, overwrite=True)
                        out.append(ev)
                        n_new += 1
                    inst.sync_info = mybir.SyncInfo(
                        on_wait=keep, on_update=list(si.on_update)
                    )
                out.append(inst)
            if changed:
                blk.instructions = out
    return n_new


class SplitDrainTileContext(TileContext):
    """The walrus build in this container rejects Drain (CTRL_NO)
    instructions carrying more than ~2 sync waits; split the tail drain's
    waits across a chain of Drain instructions, one wait each."""

    def _drain_and_barrier(self, tick_clock, wait_clock):
        nc = self.nc
        drain_inst = nc.sync.drain()
        wait_clock.add_sem_waits(
            drain_inst.ins, ScopedClock({None: tick_clock.global_clock})
        )
        si = drain_inst.ins.sync_info
        waits = list(si.on_wait) if si is not None else []
        if len(waits) > _MAX_WAITS_PER_DRAIN:
            drain_inst.ins.sync_info = mybir.SyncInfo(
                on_wait=waits[:_MAX_WAITS_PER_DRAIN], on_update=[]
            )
            for i in range(_MAX_WAITS_PER_DRAIN, len(waits), _MAX_WAITS_PER_DRAIN):
                extra = nc.sync.drain()
                extra.ins.sync_info = mybir.SyncInfo(
                    on_wait=waits[i : i + _MAX_WAITS_PER_DRAIN], on_update=[]
                )

        nc.all_engine_barrier()
        assert self.sems is not None
        popped = nc._tile_sem_poison_stack.pop()
        assert popped is self._sem_poison
        nc.clear_and_free_semaphores(list(self.sems.allocated().values()))
        nc.all_engine_barrier()


def build_nc(repeat: int = 0):
    """repeat=0: straight-line kernel. repeat>=1: whole body wrapped in a
    For_i loop run `repeat` times (only used for timing measurements)."""
    nc = bass.Bass()

    wu_hi = nc.dram_tensor("wu_hi", [128, 2, 2, G], FP8, kind="ExternalInput")
    wu_lo = nc.dram_tensor("wu_lo", [128, 2, 2, G], FP8, kind="ExternalInput")
    wx = nc.dram_tensor("wx", [1, G], BF16, kind="ExternalInput")
    w12 = nc.dram_tensor("w12", [128, 4, 2], F16, kind="ExternalInput")
    h0hi = nc.dram_tensor("h0hi", [128, 2, 2, B], FP8, kind="ExternalInput")
    h0lo = nc.dram_tensor("h0lo", [128, 2, 2, B], FP8, kind="ExternalInput")
    c0 = nc.dram_tensor("c0", [128, 4, B], F16, kind="ExternalInput")
    x0 = nc.dram_tensor("x0", [1, B], BF16, kind="ExternalInput")
    b12 = nc.dram_tensor("b12", [2, 1], F32, kind="ExternalInput")
    b2col = nc.dram_tensor("b2col", [K, 1], F32, kind="ExternalInput")
    ys1 = nc.dram_tensor("ys1", [K, B], F32, kind="ExternalOutput")
    ys2 = nc.dram_tensor("ys2", [K, B], F32, kind="ExternalOutput")

    with SplitDrainTileContext(nc) as tc:
        with contextlib.ExitStack() as ctx:
            wpool = ctx.enter_context(tc.tile_pool(name="w", bufs=1))
            hpool = ctx.enter_context(tc.tile_pool(name="h8", bufs=2))
            hlopool = ctx.enter_context(tc.tile_pool(name="h8lo", bufs=1))
            h16pool = ctx.enter_context(tc.tile_pool(name="h16", bufs=2))
            cpool = ctx.enter_context(tc.tile_pool(name="c", bufs=2))
            gpool = ctx.enter_context(tc.tile_pool(name="g", bufs=5))
            tpool = ctx.enter_context(tc.tile_pool(name="t", bufs=3))
            xpool = ctx.enter_context(tc.tile_pool(name="x", bufs=2))
            ypool = ctx.enter_context(tc.tile_pool(name="y", bufs=2))
            opool = ctx.enter_context(tc.tile_pool(name="o", bufs=1))
            zifo = ctx.enter_context(tc.tile_pool(name="zifo", bufs=2, space="PSUM"))
            zgp = ctx.enter_context(tc.tile_pool(name="zg", bufs=1, space="PSUM"))
            yps = ctx.enter_context(tc.tile_pool(name="yp", bufs=1, space="PSUM"))

            loop_cm = tc.For_i(0, repeat) if repeat else contextlib.nullcontext()
            with loop_cm:
                # --- weights + state init -------------------------------
                wu_hi_sb = wpool.tile([128, 2, 2, G], FP8, tag="wuhi")
                nc.sync.dma_start(wu_hi_sb[:, :, :, :], wu_hi[:, :, :, :])
                wx_sb = wpool.tile([1, G], BF16, tag="wx")
                nc.sync.dma_start(wx_sb[0:1, :], wx[0:1, :])
                wu_lo_sb = wpool.tile([128, 2, 2, G], FP8, tag="wulo")
                nc.scalar.dma_start(wu_lo_sb[:, :, :, :], wu_lo[:, :, :, :])
                w12_sb = wpool.tile([128, 4, 2], F16, tag="w12")
                nc.scalar.dma_start(w12_sb[:, :, :], w12[:, :, :])
                b12_sb = wpool.tile([2, 1], F32, tag="b12")
                nc.scalar.dma_start(b12_sb[:, :], b12[:, :])
                b2c_sb = wpool.tile([K, 1], F32, tag="b2col")
                nc.scalar.dma_start(b2c_sb[:, :], b2col[:, :])

                hp_prev = {}
                hp0_lo = {}
                for j in range(2):
                    hp = hpool.tile([128, 2, B], FP8, tag=f"h8_{j}")
                    nc.sync.dma_start(hp[:, :, :], h0hi[:, j, :, :])
                    hp_prev[j] = hp
                    hl = hlopool.tile([128, 2, B], FP8, tag=f"h8lo_{j}")
                    nc.scalar.dma_start(hl[:, :, :], h0lo[:, j, :, :])
                    hp0_lo[j] = hl
                c_prev = {}
                for k in range(4):
                    ct = cpool.tile([128, B], F16, tag=f"c_{k}")
                    nc.scalar.dma_start(ct[:, :], c0[:, k, :])
                    c_prev[k] = ct
                x_t = xpool.tile([1, B], BF16, tag="x")
                nc.sync.dma_start(x_t[0:1, :], x0[0:1, :])

                ys2pre = opool.tile([K, B], F32, tag="ys2pre")
                h16_prev = {}

                def yhead(tprev, h16, x_dst):
                    """dense1/dense2 matmuls + y1 sigmoid for step tprev; y1
                    also feeds back (bf16) into x_dst's row 0 + 3 row copies."""
                    for n in range(2):
                        ns = slice(n * N, (n + 1) * N)
                        yp = yps.tile([2, N], F32, tag="y")
                        for k in range(4):
                            nc.tensor.matmul(
                                yp[:, :], w12_sb[:, k, :], h16[k][:, ns],
                                start=(k == 0), stop=(k == 3),
                            )
                        yr1 = ypool.tile([1, N], F32, tag="yr1")
                        nc.scalar.activation(
                            yr1[:, :], yp[0:1, :], AF.Sigmoid,
                            bias=b12_sb[0:1, 0:1],
                        )
                        nc.sync.dma_start(ys1[tprev:tprev + 1, ns], yr1[:, :])
                        if x_dst is not None:
                            nc.vector.tensor_copy(x_dst[0:1, ns], yr1[:, :])
                        yr2 = ypool.tile([2, N], F32, tag="yr2")
                        nc.vector.tensor_copy(yr2[:, :], yp[0:2, :])
                        nc.sync.dma_start(ys2pre[tprev:tprev + 1, ns], yr2[1:2, :])

                # --- decode steps ---------------------------------------
                for t in range(K):
                    if t > 0:
                        x_t = xpool.tile([1, B], BF16, tag="x")
                        yhead(t - 1, h16_prev, x_t)
                    h16_new, c_new, hp_new = {}, {}, {}
                    for k in range(4):
                        mi, mf, mo, mg = k, 4 + k, 12 + k, 8 + k
                        o_tiles = {}
                        cn = cpool.tile([128, B], F16, tag=f"c_{k}")
                        last = k == 3
                        if last:
                            # k3's tail runs per batch-half, n1 first, so the
                            # cross-step chain (tch -> h16 -> h8/y -> x ->
                            # closers) rides the short n0 half at step end
                            hn = h16pool.tile([128, B], F16, tag=f"h16_{k}")
                        nsl = [slice(0, N), slice(N, 2 * N)]
                        # both batch halves' PSUM tiles live at once so every
                        # weight tile is consumed by two back-to-back matmuls
                        # (the PE skips the ~180ns stationary reload when
                        # consecutive matmuls share weights)
                        zifo_n = [zifo.tile([128, 3 * N], F32, tag="zifo",
                                            name=f"zifo_{n}") for n in range(2)]
                        ifo_ms = [(mi, 0), (mf, 1), (mo, 2)]
                        for m, q in ifo_ms:
                            ms = slice(m * 128, (m + 1) * 128)
                            cs = slice(q * N, (q + 1) * N)
                            for j in range(2):
                                for n in range(2):
                                    nc.tensor.matmul(
                                        zifo_n[n][:, cs], wu_hi_sb[:, j, :, ms],
                                        hp_prev[j][:, :, nsl[n]],
                                        start=(j == 0), stop=False, perf_mode=DR,
                                    )
                            if t == 0:
                                for j in range(2):
                                    for n in range(2):
                                        nc.tensor.matmul(
                                            zifo_n[n][:, cs], wu_hi_sb[:, j, :, ms],
                                            hp0_lo[j][:, :, nsl[n]],
                                            start=False, stop=False, perf_mode=DR,
                                        )
                            if t < N_WLO:
                                for j in range(2):
                                    for n in range(2):
                                        nc.tensor.matmul(
                                            zifo_n[n][:, cs], wu_lo_sb[:, j, :, ms],
                                            hp_prev[j][:, :, nsl[n]],
                                            start=False, stop=False, perf_mode=DR,
                                        )
                        # rank-1 x closers (bf16), n-paired per weight row.
                        # Gate bias is structurally zero (asserted host-side).
                        for m, q in ifo_ms:
                            ms = slice(m * 128, (m + 1) * 128)
                            cs = slice(q * N, (q + 1) * N)
                            for n in range(2):
                                nc.tensor.matmul(
                                    zifo_n[n][:, cs], wx_sb[0:1, ms],
                                    x_t[0:1, nsl[n]], start=False, stop=True,
                                )
                        # g gate: single zg bank, n-serial (tanh frees it)
                        gt = {}
                        msg = slice(mg * 128, (mg + 1) * 128)
                        for n in range(2):
                            zg_t = zgp.tile([128, N], F32, tag="zg")
                            for j in range(2):
                                nc.tensor.matmul(
                                    zg_t[:, :], wu_hi_sb[:, j, :, msg],
                                    hp_prev[j][:, :, nsl[n]],
                                    start=(j == 0), stop=False, perf_mode=DR,
                                )
                                if t == 0:
                                    nc.tensor.matmul(
                                        zg_t[:, :], wu_hi_sb[:, j, :, msg],
                                        hp0_lo[j][:, :, nsl[n]],
                                        start=False, stop=False, perf_mode=DR,
                                    )
                                if t < N_WLO:
                                    nc.tensor.matmul(
                                        zg_t[:, :], wu_lo_sb[:, j, :, msg],
                                        hp_prev[j][:, :, nsl[n]],
                                        start=False, stop=False, perf_mode=DR,
                                    )
                            nc.tensor.matmul(
                                zg_t[:, :], wx_sb[0:1, msg], x_t[0:1, nsl[n]],
                                start=False, stop=True,
                            )
                            g_t = gpool.tile([128, N], F16, tag="g", name=f"g_{n}")
                            nc.scalar.activation(
                                g_t[:, :], zg_t[:, :], AF.Tanh, scale=1.0 / SC
                            )
                            gt[n] = g_t
                        for n in ((1, 0) if last else (0, 1)):
                            ns = nsl[n]
                            ifo = gpool.tile([128, 3 * N], F16, tag="ifo",
                                             name=f"ifo_{n}")
                            nc.scalar.activation(
                                ifo[:, :], zifo_n[n][:, :], AF.Sigmoid,
                                scale=1.0 / SC
                            )
                            t2 = tpool.tile([128, N], F16, tag="t2")
                            nc.vector.tensor_mul(t2[:, :], ifo[:, 0:N], gt[n][:, :])
                            t1 = tpool.tile([128, N], F16, tag="t1")
                            nc.vector.tensor_mul(
                                t1[:, :], ifo[:, N:2 * N], c_prev[k][:, ns]
                            )
                            nc.vector.tensor_add(cn[:, ns], t1[:, :], t2[:, :])
                            o_tiles[n] = ifo
                            if last:
                                tch3 = tpool.tile([128, N], F16, tag="tch3")
                                nc.scalar.activation(tch3[:, :], cn[:, ns], AF.Tanh)
                                nc.vector.tensor_mul(
                                    hn[:, ns], ifo[:, 2 * N:3 * N], tch3[:, :]
                                )
                                if t < K - 1:
                                    nc.vector.tensor_scalar_mul(
                                        hp_new[1][:, 1, ns], hn[:, ns], SH
                                    )
                        c_new[k] = cn
                        if not last:
                            tch = tpool.tile([128, B], F16, tag="tch")
                            nc.scalar.activation(tch[:, :], cn[:, :], AF.Tanh)
                            hn = h16pool.tile([128, B], F16, tag=f"h16_{k}")
                            for n in range(2):
                                ns = slice(n * N, (n + 1) * N)
                                nc.vector.tensor_mul(
                                    hn[:, ns], o_tiles[n][:, 2 * N:3 * N], tch[:, ns]
                                )
                            if t < K - 1:
                                j, i = k // 2, k % 2
                                if i == 0:
                                    hp_new[j] = hpool.tile(
                                        [128, 2, B], FP8, tag=f"h8_{j}",
                                        name=f"h8n_{j}"
                                    )
                                nc.vector.tensor_scalar_mul(
                                    hp_new[j][:, i, :], hn[:, :], SH
                                )
                        h16_new[k] = hn
                    h16_prev, c_prev, hp_prev = h16_new, c_new, hp_new

                yhead(K - 1, h16_prev, None)

                # --- batched elu tail: y2 = relu(p) + exp(min(p,0)) - 1 --
                pb = opool.tile([K, B], F32, tag="elu_p")
                nc.scalar.activation(
                    pb[:, :], ys2pre[:, :], AF.Identity, bias=b2c_sb[:, 0:1]
                )
                r = opool.tile([K, B], F32, tag="elu_r")
                nc.scalar.activation(r[:, :], pb[:, :], AF.Relu)
                neg = opool.tile([K, B], F32, tag="elu_n")
                nc.vector.tensor_sub(neg[:, :], pb[:, :], r[:, :])
                e = opool.tile([K, B], F32, tag="elu_e")
                nc.scalar.activation(e[:, :], neg[:, :], AF.Exp)
                s = opool.tile([K, B], F32, tag="elu_s")
                nc.vector.tensor_add(s[:, :], r[:, :], e[:, :])
                y2f = opool.tile([K, B], F32, tag="elu_y")
                nc.vector.tensor_scalar_add(y2f[:, :], s[:, :], -1.0)
                nc.sync.dma_start(ys2[:, :], y2f[:, :])

    _split_waits(nc)
    return nc


def make_in_map(initial, encoder_hidden, encoder_cell, Wx, Wu, b, w1, b1, w2, b2):
    """Per-core input dict from this core's batch shard (numpy fp32 arrays)."""
    import ml_dtypes
    e4 = ml_dtypes.float8_e4m3
    bf = ml_dtypes.bfloat16

    def pair_layout(w_rows):  # [D, F] -> [128, 2, 2, F]  (row d = (2j+i)*128+p)
        f = w_rows.shape[1]
        return np.ascontiguousarray(
            w_rows.reshape(2, 2, 128, f).transpose(2, 0, 1, 3)
        )

    assert not np.any(b), "gate bias must be zero (no bias closers built)"
    Wus = (Wu * SW).astype(np.float32)
    wu_hi = Wus.astype(e4)
    wu_lo = (Wus - wu_hi.astype(np.float32)).astype(e4)

    hT = np.ascontiguousarray(encoder_hidden.T).astype(np.float32)
    hs = hT * SH
    h_hi = hs.astype(e4)
    h_lo = (hs - h_hi.astype(np.float32)).astype(e4)

    cT = np.ascontiguousarray(encoder_cell.T)
    w12f = np.concatenate([w1, w2], axis=1)  # [D, 2]

    return {
        "wu_hi": pair_layout(wu_hi),
        "wu_lo": pair_layout(wu_lo),
        "wx": np.ascontiguousarray((Wx * SC).astype(bf).reshape(1, G)),
        "w12": np.ascontiguousarray(
            w12f.reshape(4, 128, 2).transpose(1, 0, 2)
        ).astype(np.float16),
        "h0hi": pair_layout(h_hi),
        "h0lo": pair_layout(h_lo),
        "c0": np.ascontiguousarray(
            cT.reshape(4, 128, B).transpose(1, 0, 2)
        ).astype(np.float16),
        "x0": np.ascontiguousarray(initial[:, 0, :].T.astype(bf).reshape(1, B)),
        "b12": np.array([[np.float32(b1[0])], [np.float32(b2[0])]], dtype=np.float32),
        "b2col": np.full((K, 1), np.float32(b2[0]), dtype=np.float32),
    }


_CACHE = {}


def _get_nc():
    if "nc" not in _CACHE:
        _CACHE["nc"] = build_nc(repeat=0)
    return _CACHE["nc"]


def kernel(initial, encoder_hidden, encoder_cell, Wx, Wu, b, w1, b1, w2, b2):
    from concourse import bass_utils

    initial = np.asarray(initial, dtype=np.float32)
    encoder_hidden = np.asarray(encoder_hidden, dtype=np.float32)
    encoder_cell = np.asarray(encoder_cell, dtype=np.float32)
    Wx = np.asarray(Wx, dtype=np.float32)
    Wu = np.asarray(Wu, dtype=np.float32)
    b = np.asarray(b, dtype=np.float32)
    w1 = np.asarray(w1, dtype=np.float32)
    b1 = np.asarray(b1, dtype=np.float32)
    w2 = np.asarray(w2, dtype=np.float32)
    b2 = np.asarray(b2, dtype=np.float32)

    nc = _get_nc()
    in_maps = []
    for c in range(NCORES):
        sl = slice(c * B, (c + 1) * B)
        in_maps.append(
            make_in_map(initial[sl], encoder_hidden[sl], encoder_cell[sl],
                        Wx, Wu, b, w1, b1, w2, b2)
        )
    res = bass_utils.run_bass_kernel_spmd(nc, in_maps, core_ids=list(range(NCORES)))
    out1 = np.concatenate([res.results[c]["ys1"].T for c in range(NCORES)], axis=0)
    out2 = np.concatenate([res.results[c]["ys2"].T for c in range(NCORES)], axis=0)
    return (np.ascontiguousarray(out1, dtype=np.float32),
            np.ascontiguousarray(out2, dtype=np.float32))
